# revision 24
# baseline (speedup 1.0000x reference)
"""Multi-head attention (softmax over query axis) on 8 TRN2 NeuronCores.

Data-parallel over batch: core b computes batch element b entirely locally
(B == n_cores == 8), so no collectives are needed.

Math (per batch element, x: [P, D]):
    qkv = x @ W_qkv ; q,k,v heads of dim DH=64
    dots = q @ k^T * SCALE              [h, P, P]
    A = softmax(dots, axis=-2)          (normalized over the QUERY axis i)
    out = (A @ v per head) @ W_out + b_out

Device strategy:
    xT [D, P] pre-transposed on host, W_q pre-scaled by SCALE, inputs bf16.
    A: qT/kT [c, p] head-pair tiles; B: v [p, c].
    C: dots_T[j, i] per head via 64-row-tiled PE head pairs (f32 psum).
    softmax over i (free axis): ScalarE Exp with accum_out row sums;
    1/sum folded into V rows (tiny per-partition scalar mul).
    D: attn_out_T[dh, i] via 64-col-tiled PE head pairs accumulating over j,
    landing directly in [c, p] layout for the out-projection E.

    The ScalarE exp stream is the critical resource (~92us); emission
    order interleaves C units with D/proj/B units at sub-microsecond
    granularity so the PE never idles long enough to re-throttle (HAM).
"""
import numpy as np

import concourse.bass as bass
import concourse.tile as tile
from concourse import bacc, mybir
from concourse.bass_utils import run_bass_kernel_spmd

B, P, D = 8, 1024, 512
H, DH = 8, 64
SCALE = DH ** -0.5
F32 = mybir.dt.float32
BF16 = mybir.dt.bfloat16
NCORES = 8

KT = D // 128        # 4 contraction k-tiles over D
PT = P // 128        # 8 p-tiles
NPAIR = H // 2       # 4 head pairs
IH = P // 512        # 2 i-halves (PSUM bank = 512 f32)


def build():
    nc = bacc.Bacc(trn_type="TRN2")
    xT_ext = nc.declare_dram_parameter("xT", [D, P], BF16, isOutput=False)
    wq_ext = nc.declare_dram_parameter("wq", [D, D], BF16, isOutput=False)
    wk_ext = nc.declare_dram_parameter("wk", [D, D], BF16, isOutput=False)
    wv_ext = nc.declare_dram_parameter("wv", [D, D], BF16, isOutput=False)
    wo_ext = nc.declare_dram_parameter("wo", [D, D], BF16, isOutput=False)
    out_ext = nc.declare_dram_parameter("out", [P, D], F32, isOutput=True)

    with tile.TileContext(nc) as tc:
        with (
            tc.tile_pool(name="persist", bufs=1) as pp,
            tc.tile_pool(name="aT", bufs=2) as ap_,
            tc.tile_pool(name="vp", bufs=2) as vpp,
            tc.tile_pool(name="sums", bufs=2) as sp,
            tc.tile_pool(name="osb", bufs=2) as op_,
            tc.tile_pool(name="ps_main", bufs=2, space="PSUM") as ps_main,
            tc.tile_pool(name="ps_c", bufs=2, space="PSUM") as ps_c,
            tc.tile_pool(name="ps_d", bufs=1, space="PSUM") as ps_d,
        ):
            # ---- input DMA (two HWDGE queues: sync + scalar) ----
            xT = [pp.tile([128, P], BF16, name=f"xT{k}", tag=f"xT{k}") for k in range(KT)]
            wq = [pp.tile([128, D], BF16, name=f"wq{k}", tag=f"wq{k}") for k in range(KT)]
            wk = [pp.tile([128, D], BF16, name=f"wk{k}", tag=f"wk{k}") for k in range(KT)]
            wv = [pp.tile([128, D], BF16, name=f"wv{k}", tag=f"wv{k}") for k in range(KT)]
            wo = [pp.tile([128, D], BF16, name=f"wo{k}", tag=f"wo{k}") for k in range(KT)]
            for k in range(KT):
                r = slice(k * 128, (k + 1) * 128)
                nc.sync.dma_start(out=xT[k], in_=xT_ext[r, :])
                nc.scalar.dma_start(out=wq[k], in_=wq_ext[r, :])
            for k in range(KT):
                r = slice(k * 128, (k + 1) * 128)
                nc.scalar.dma_start(out=wk[k], in_=wk_ext[r, :])
                nc.sync.dma_start(out=wv[k], in_=wv_ext[r, :])
                nc.scalar.dma_start(out=wo[k], in_=wo_ext[r, :])

            # persistent activation storage
            qT = [pp.tile([128, P], BF16, name=f"qT{c}", tag=f"qT{c}") for c in range(NPAIR)]
            kTt = [pp.tile([128, P], BF16, name=f"kT{c}", tag=f"kT{c}") for c in range(NPAIR)]
            vt = [pp.tile([128, D], F32, name=f"v{p}", tag=f"v{p}") for p in range(PT)]
            aoT = [pp.tile([128, P], BF16, name=f"aoT{c}", tag=f"aoT{c}") for c in range(NPAIR)]

            def proj_qk(w, ct, dst):
                """dst [128,P] = (x @ W)^T c-slice. Yields per i-half."""
                for ih in range(IH):
                    ps = ps_main.tile([128, 512], F32, name="ps_main", tag="ps_main")
                    for k in range(KT):
                        nc.tensor.matmul(
                            out=ps,
                            lhsT=w[k][:, ct * 128:(ct + 1) * 128],
                            rhs=xT[k][:, ih * 512:(ih + 1) * 512],
                            start=(k == 0), stop=(k == KT - 1),
                        )
                    nc.vector.tensor_copy(dst[:, ih * 512:(ih + 1) * 512], ps)
                    yield

            def proj_v(pt):
                """vt[pt] [128, D] = x p-tile @ W_v. Yields once."""
                ps = ps_main.tile([128, 512], F32, name="ps_main", tag="ps_main")
                for k in range(KT):
                    nc.tensor.matmul(
                        out=ps,
                        lhsT=xT[k][:, pt * 128:(pt + 1) * 128],
                        rhs=wv[k],
                        start=(k == 0), stop=(k == KT - 1),
                    )
                nc.vector.tensor_copy(vt[pt], ps)
                yield

            pair_data = {}

            def attn_pair(pr):
                """dots + exp + row sums for head pair pr. Yields per (jt, h)."""
                a_t = [[ap_.tile([128, P], BF16, name=f"a{h}_{jt}", tag=f"a{h}_{jt}")
                        for jt in range(PT)] for h in range(2)]
                sums = [sp.tile([128, PT], F32, name=f"sums{h}", tag=f"sums{h}")
                        for h in range(2)]
                pair_data[pr] = (a_t, sums)
                for h in range(2):
                    for jt in range(PT):
                        hp = slice(h * 64, (h + 1) * 64)
                        ps = ps_c.tile([128, P], F32, name="ps_c", tag="ps_c")
                        for ih in range(IH):
                            nc.tensor.matmul(
                                out=ps[:, ih * 512:(ih + 1) * 512],
                                lhsT=kTt[pr][hp, jt * 128:(jt + 1) * 128],
                                rhs=qT[pr][hp, ih * 512:(ih + 1) * 512],
                                start=True, stop=True,
                                tile_position=(h * 64, 0),
                            )
                        nc.scalar.activation(
                            out=a_t[h][jt],
                            in_=ps,
                            func=mybir.ActivationFunctionType.Exp,
                            accum_out=sums[h][:, jt:jt + 1],
                        )
                        yield

            vp_data = {}

            def prep_av(pr, h):
                """reciprocal + V-row scaling for (pair, head) (cheap, DVE)."""
                a_t, sums = pair_data[pr]
                rr = sp.tile([128, PT], F32, name=f"recip{h}", tag=f"recip{h}")
                nc.vector.reciprocal(rr, sums[h])
                if h == 0:
                    vp_data[pr] = [vpp.tile([128, 128], BF16, name=f"vp{jt}", tag=f"vp{jt}")
                                   for jt in range(PT)]
                vp = vp_data[pr]
                hc = (2 * pr + h) * 64
                for jt in range(PT):
                    nc.vector.tensor_scalar_mul(
                        vp[jt][:, h * 64:(h + 1) * 64],
                        vt[jt][:, hc:hc + 64],
                        rr[:, jt:jt + 1],
                    )
                return vp

            def attn_av(pr, h):
                """contract A^T (head h) with V': aoT[pr] half. Yields per 4 MMs."""
                a_t, _ = pair_data[pr]
                vp = vp_data[pr]
                hp = slice(h * 64, (h + 1) * 64)
                for ih in range(IH):
                    psd = ps_d.tile([128, 512], F32, name=f"ps_d{h}", tag=f"ps_d{h}")
                    for jt in range(PT):
                        nc.tensor.matmul(
                            out=psd[hp, :],
                            lhsT=vp[jt][:, h * 64:(h + 1) * 64],
                            rhs=a_t[h][jt][:, ih * 512:(ih + 1) * 512],
                            start=(jt == 0), stop=(jt == PT - 1),
                            tile_position=(0, h * 64),
                        )
                        if jt % 4 == 3:
                            yield
                    nc.vector.tensor_copy(
                        aoT[pr][hp, ih * 512:(ih + 1) * 512], psd[hp, :]
                    )
                    yield

            def chain(*gens):
                for g in gens:
                    yield from g

            def interleave(main, filler, ms=2, fs=2):
                """Emit ms units of main, then fs units of filler, repeating.

                Block sizes keep the PE in one tiling mode for several
                matmuls at a time (mode switches drain the PE array).
                """
                while True:
                    done = 0
                    for g, n in ((main, ms), (filler, fs)):
                        try:
                            for _ in range(n):
                                next(g)
                        except StopIteration:
                            done += 1
                    if done == 2:
                        return

            # ---- emission schedule ----
            # slot 0: C(0)  ||  B + prefetch qk(1), qk(2)
            # slot 1: C(1)  ||  D(0) + prefetch qk(3)
            # slot 2: C(2)  ||  D(1)
            # slot 3: C(3)  ||  D(2)
            # tail  : D(3), E
            def out_proj(pts):
                for pt in pts:
                    ps = ps_main.tile([128, 512], F32, name="ps_main", tag="ps_main")
                    for ct in range(KT):
                        nc.tensor.matmul(
                            out=ps,
                            lhsT=aoT[ct][:, pt * 128:(pt + 1) * 128],
                            rhs=wo[ct],
                            start=(ct == 0), stop=(ct == KT - 1),
                        )
                    ot = op_.tile([128, 512], F32, name="osb", tag="osb")
                    nc.vector.tensor_copy(ot, ps)
                    eng = nc.sync if pt % 2 == 0 else nc.scalar
                    eng.dma_start(out=out_ext[pt * 128:(pt + 1) * 128, :], in_=ot)
                    yield

            def pair_filler(pr):
                """Filler for slot pr (C of pair pr runs h0 then h1):
                D of pair pr-1: head 0 first (ready), then head 1."""
                prep_av(pr - 1, 0)
                yield from attn_av(pr - 1, 0)
                prep_av(pr - 1, 1)
                yield from attn_av(pr - 1, 1)

            for g in chain(proj_qk(wq, 0, qT[0]), proj_qk(wk, 0, kTt[0])):
                pass
            interleave(
                attn_pair(0),
                chain(*[proj_v(pt) for pt in range(PT)],
                      proj_qk(wq, 1, qT[1]), proj_qk(wk, 1, kTt[1]),
                      proj_qk(wq, 2, qT[2]), proj_qk(wk, 2, kTt[2])),
            )
            interleave(
                attn_pair(1),
                chain(pair_filler(1),
                      proj_qk(wq, 3, qT[3]), proj_qk(wk, 3, kTt[3])),
            )
            interleave(attn_pair(2), pair_filler(2))
            interleave(attn_pair(3), pair_filler(3))
            # tail: D(3) h0/h1 with the out-projection interleaved per i-half
            prep_av(3, 0)
            for _ in attn_av(3, 0):
                pass
            prep_av(3, 1)
            for _ in attn_av(3, 1):
                pass
            for _ in out_proj(range(PT)):
                pass


    nc.finalize()
    return nc


_NC = None


def _get_nc():
    global _NC
    if _NC is None:
        _NC = build()
    return _NC


def run(x, W_qkv, W_out, b_out, trace=False, tmpdir=None):
    import ml_dtypes

    x = np.asarray(x, dtype=np.float32)
    W_qkv = np.asarray(W_qkv, dtype=np.float32)
    W_out = np.asarray(W_out, dtype=np.float32)
    b_out = np.asarray(b_out, dtype=np.float32)

    bf = ml_dtypes.bfloat16
    wq_h = (np.ascontiguousarray(W_qkv[:, :D]) * np.float32(SCALE)).astype(bf)
    wk_h = np.ascontiguousarray(W_qkv[:, D:2 * D]).astype(bf)
    wv_h = np.ascontiguousarray(W_qkv[:, 2 * D:]).astype(bf)
    wo_h = W_out.astype(bf)
    in_maps = [
        {
            "xT": np.ascontiguousarray(x[b].T).astype(bf),
            "wq": wq_h, "wk": wk_h, "wv": wv_h, "wo": wo_h,
        }
        for b in range(NCORES)
    ]
    nc = _get_nc()
    res = run_bass_kernel_spmd(
        nc, in_maps, core_ids=list(range(NCORES)), trace=trace, tmpdir=tmpdir
    )
    out = np.stack([res.results[b]["out"] for b in range(NCORES)], axis=0)
    out = out + b_out[None, None, :]
    return out.astype(np.float32), res


def kernel(x, W_qkv, W_out, b_out):
    out, _ = run(x, W_qkv, W_out, b_out, trace=False)
    return out


# revision 25
# speedup vs baseline: 1.1766x; 1.1766x over previous
"""Multi-head attention (softmax over query axis) on 8 TRN2 NeuronCores.

Data-parallel over batch: core b computes batch element b entirely locally
(B == n_cores == 8), so no collectives are needed.

Math (per batch element, x: [P, D]):
    qkv = x @ W_qkv ; q,k,v heads of dim DH=64
    dots = q @ k^T * SCALE              [h, P, P]
    A = softmax(dots, axis=-2)          (normalized over the QUERY axis i)
    out = (A @ v per head) @ W_out + b_out

Device strategy:
    xT [D, P] pre-transposed on host, W_q pre-scaled by SCALE, inputs bf16.
    A: qT/kT [c, p] head-pair tiles; B: v [p, c].
    C: dots_T[j, i] per head via 64-row-tiled PE head pairs (f32 psum).
    softmax over i (free axis): ScalarE Exp with accum_out row sums;
    1/sum folded into V rows (tiny per-partition scalar mul).
    D: attn_out_T[dh, i] via 64-col-tiled PE head pairs accumulating over j,
    landing directly in [c, p] layout for the out-projection E.

    The ScalarE exp stream is the critical resource (~92us); emission
    order interleaves C units with D/proj/B units at sub-microsecond
    granularity so the PE never idles long enough to re-throttle (HAM).
"""
import numpy as np

import concourse.bass as bass
import concourse.tile as tile
from concourse import bacc, mybir
from concourse.bass_utils import run_bass_kernel_spmd

B, P, D = 8, 1024, 512
H, DH = 8, 64
SCALE = DH ** -0.5
F32 = mybir.dt.float32
BF16 = mybir.dt.bfloat16
NCORES = 8

KT = D // 128        # 4 contraction k-tiles over D
PT = P // 128        # 8 p-tiles
NPAIR = H // 2       # 4 head pairs
IH = P // 512        # 2 i-halves (PSUM bank = 512 f32)


def build():
    nc = bacc.Bacc(trn_type="TRN2")
    xT_ext = nc.declare_dram_parameter("xT", [D, P], BF16, isOutput=False)
    wq_ext = nc.declare_dram_parameter("wq", [D, D], BF16, isOutput=False)
    wk_ext = nc.declare_dram_parameter("wk", [D, D], BF16, isOutput=False)
    wv_ext = nc.declare_dram_parameter("wv", [D, D], BF16, isOutput=False)
    wo_ext = nc.declare_dram_parameter("wo", [D, D], BF16, isOutput=False)
    out_ext = nc.declare_dram_parameter("out", [P, D], F32, isOutput=True)

    with tile.TileContext(nc) as tc:
        with (
            tc.tile_pool(name="persist", bufs=1) as pp,
            tc.tile_pool(name="aT", bufs=2) as ap_,
            tc.tile_pool(name="vp", bufs=2) as vpp,
            tc.tile_pool(name="sums", bufs=2) as sp,
            tc.tile_pool(name="osb", bufs=2) as op_,
            tc.tile_pool(name="ps_main", bufs=2, space="PSUM") as ps_main,
            tc.tile_pool(name="ps_c", bufs=2, space="PSUM") as ps_c,
            tc.tile_pool(name="ps_d", bufs=1, space="PSUM") as ps_d,
        ):
            # ---- input DMA (two HWDGE queues: sync + scalar) ----
            xT = [pp.tile([128, P], BF16, name=f"xT{k}", tag=f"xT{k}") for k in range(KT)]
            wq = [pp.tile([128, D], BF16, name=f"wq{k}", tag=f"wq{k}") for k in range(KT)]
            wk = [pp.tile([128, D], BF16, name=f"wk{k}", tag=f"wk{k}") for k in range(KT)]
            wv = [pp.tile([128, D], BF16, name=f"wv{k}", tag=f"wv{k}") for k in range(KT)]
            wo = [pp.tile([128, D], BF16, name=f"wo{k}", tag=f"wo{k}") for k in range(KT)]
            for k in range(KT):
                r = slice(k * 128, (k + 1) * 128)
                nc.sync.dma_start(out=xT[k], in_=xT_ext[r, :])
                nc.scalar.dma_start(out=wq[k], in_=wq_ext[r, :])
            for k in range(KT):
                r = slice(k * 128, (k + 1) * 128)
                nc.scalar.dma_start(out=wk[k], in_=wk_ext[r, :])
                nc.sync.dma_start(out=wv[k], in_=wv_ext[r, :])
                nc.scalar.dma_start(out=wo[k], in_=wo_ext[r, :])

            # persistent activation storage
            qT = [pp.tile([128, P], BF16, name=f"qT{c}", tag=f"qT{c}") for c in range(NPAIR)]
            kTt = [pp.tile([128, P], BF16, name=f"kT{c}", tag=f"kT{c}") for c in range(NPAIR)]
            vt = [pp.tile([128, D], F32, name=f"v{p}", tag=f"v{p}") for p in range(PT)]
            aoT = [pp.tile([128, P], BF16, name=f"aoT{c}", tag=f"aoT{c}") for c in range(NPAIR)]

            def proj_qk(w, ct, dst):
                """dst [128,P] = (x @ W)^T c-slice. Yields per i-half."""
                for ih in range(IH):
                    ps = ps_main.tile([128, 512], F32, name="ps_main", tag="ps_main")
                    for k in range(KT):
                        nc.tensor.matmul(
                            out=ps,
                            lhsT=w[k][:, ct * 128:(ct + 1) * 128],
                            rhs=xT[k][:, ih * 512:(ih + 1) * 512],
                            start=(k == 0), stop=(k == KT - 1),
                        )
                    nc.vector.tensor_copy(dst[:, ih * 512:(ih + 1) * 512], ps)
                    yield

            def proj_v(pt):
                """vt[pt] [128, D] = x p-tile @ W_v. Yields once."""
                ps = ps_main.tile([128, 512], F32, name="ps_main", tag="ps_main")
                for k in range(KT):
                    nc.tensor.matmul(
                        out=ps,
                        lhsT=xT[k][:, pt * 128:(pt + 1) * 128],
                        rhs=wv[k],
                        start=(k == 0), stop=(k == KT - 1),
                    )
                nc.vector.tensor_copy(vt[pt], ps)
                yield

            pair_data = {}

            def attn_pair(pr):
                """dots + exp + row sums for head pair pr. Yields per (jt, h)."""
                a_t = [[ap_.tile([128, P], BF16, name=f"a{h}_{jt}", tag=f"a{h}_{jt}")
                        for jt in range(PT)] for h in range(2)]
                sums = [sp.tile([128, PT], F32, name=f"sums{h}", tag=f"sums{h}")
                        for h in range(2)]
                pair_data[pr] = (a_t, sums)
                for jt in range(PT):
                    for h in range(2):
                        hp = slice(h * 64, (h + 1) * 64)
                        ps = ps_c.tile([128, P], F32, name="ps_c", tag="ps_c")
                        for ih in range(IH):
                            nc.tensor.matmul(
                                out=ps[:, ih * 512:(ih + 1) * 512],
                                lhsT=kTt[pr][hp, jt * 128:(jt + 1) * 128],
                                rhs=qT[pr][hp, ih * 512:(ih + 1) * 512],
                                start=True, stop=True,
                                tile_position=(h * 64, 0),
                            )
                        nc.scalar.activation(
                            out=a_t[h][jt],
                            in_=ps,
                            func=mybir.ActivationFunctionType.Exp,
                            accum_out=sums[h][:, jt:jt + 1],
                        )
                        yield

            def prep_av(pr):
                """reciprocal + V-row scaling for pair pr (cheap, DVE)."""
                a_t, sums = pair_data[pr]
                rr = sp.tile([128, 2, PT], F32, name="recip", tag="recip")
                for h in range(2):
                    nc.vector.reciprocal(rr[:, h, :], sums[h])
                vp = [vpp.tile([128, 128], BF16, name=f"vp{jt}", tag=f"vp{jt}")
                      for jt in range(PT)]
                for jt in range(PT):
                    for h in range(2):
                        hc = (2 * pr + h) * 64
                        nc.vector.tensor_scalar_mul(
                            vp[jt][:, h * 64:(h + 1) * 64],
                            vt[jt][:, hc:hc + 64],
                            rr[:, h, jt:jt + 1],
                        )
                return vp

            def attn_av(pr, vp):
                """contract A^T with V' (col-tiled head pair): aoT[pr]."""
                a_t, _ = pair_data[pr]
                for ih in range(IH):
                    psd = [ps_d.tile([128, 512], F32, name=f"ps_d{h}", tag=f"ps_d{h}")
                           for h in range(2)]
                    for jt in range(PT):
                        for h in range(2):
                            nc.tensor.matmul(
                                out=psd[h][h * 64:(h + 1) * 64, :],
                                lhsT=vp[jt][:, h * 64:(h + 1) * 64],
                                rhs=a_t[h][jt][:, ih * 512:(ih + 1) * 512],
                                start=(jt == 0), stop=(jt == PT - 1),
                                tile_position=(0, h * 64),
                            )
                        if jt % 2 == 1:
                            yield
                    for h in range(2):
                        hp = slice(h * 64, (h + 1) * 64)
                        nc.vector.tensor_copy(
                            aoT[pr][hp, ih * 512:(ih + 1) * 512], psd[h][hp, :]
                        )
                    yield

            def chain(*gens):
                for g in gens:
                    yield from g

            def interleave(main, filler, ms=2, fs=2):
                """Emit ms units of main, then fs units of filler, repeating.

                Block sizes keep the PE in one tiling mode for several
                matmuls at a time (mode switches drain the PE array).
                """
                while True:
                    done = 0
                    for g, n in ((main, ms), (filler, fs)):
                        try:
                            for _ in range(n):
                                next(g)
                        except StopIteration:
                            done += 1
                    if done == 2:
                        return

            # ---- emission schedule ----
            # slot 0: C(0)  ||  B + prefetch qk(1), qk(2)
            # slot 1: C(1)  ||  D(0) + prefetch qk(3)
            # slot 2: C(2)  ||  D(1)
            # slot 3: C(3)  ||  D(2)
            # tail  : D(3), E
            def out_proj(pts):
                for pt in pts:
                    ps = ps_main.tile([128, 512], F32, name="ps_main", tag="ps_main")
                    for ct in range(KT):
                        nc.tensor.matmul(
                            out=ps,
                            lhsT=aoT[ct][:, pt * 128:(pt + 1) * 128],
                            rhs=wo[ct],
                            start=(ct == 0), stop=(ct == KT - 1),
                        )
                    ot = op_.tile([128, 512], F32, name="osb", tag="osb")
                    nc.vector.tensor_copy(ot, ps)
                    eng = nc.sync if pt % 2 == 0 else nc.scalar
                    eng.dma_start(out=out_ext[pt * 128:(pt + 1) * 128, :], in_=ot)
                    yield

            for g in chain(proj_qk(wq, 0, qT[0]), proj_qk(wk, 0, kTt[0])):
                pass
            interleave(
                attn_pair(0),
                chain(*[proj_v(pt) for pt in range(PT)],
                      proj_qk(wq, 1, qT[1]), proj_qk(wk, 1, kTt[1]),
                      proj_qk(wq, 2, qT[2]), proj_qk(wk, 2, kTt[2])),
            )
            vp0 = prep_av(0)
            interleave(
                attn_pair(1),
                chain(attn_av(0, vp0),
                      proj_qk(wq, 3, qT[3]), proj_qk(wk, 3, kTt[3])),
            )
            vp1 = prep_av(1)
            interleave(attn_pair(2), attn_av(1, vp1))
            vp2 = prep_av(2)
            interleave(attn_pair(3), attn_av(2, vp2))
            vp3 = prep_av(3)
            for _ in attn_av(3, vp3):
                pass
            for _ in out_proj(range(PT)):
                pass


    nc.finalize()
    return nc


_NC = None


def _get_nc():
    global _NC
    if _NC is None:
        _NC = build()
    return _NC


def run(x, W_qkv, W_out, b_out, trace=False, tmpdir=None):
    import ml_dtypes

    x = np.asarray(x, dtype=np.float32)
    W_qkv = np.asarray(W_qkv, dtype=np.float32)
    W_out = np.asarray(W_out, dtype=np.float32)
    b_out = np.asarray(b_out, dtype=np.float32)

    bf = ml_dtypes.bfloat16
    wq_h = (np.ascontiguousarray(W_qkv[:, :D]) * np.float32(SCALE)).astype(bf)
    wk_h = np.ascontiguousarray(W_qkv[:, D:2 * D]).astype(bf)
    wv_h = np.ascontiguousarray(W_qkv[:, 2 * D:]).astype(bf)
    wo_h = W_out.astype(bf)
    in_maps = [
        {
            "xT": np.ascontiguousarray(x[b].T).astype(bf),
            "wq": wq_h, "wk": wk_h, "wv": wv_h, "wo": wo_h,
        }
        for b in range(NCORES)
    ]
    nc = _get_nc()
    res = run_bass_kernel_spmd(
        nc, in_maps, core_ids=list(range(NCORES)), trace=trace, tmpdir=tmpdir
    )
    out = np.stack([res.results[b]["out"] for b in range(NCORES)], axis=0)
    out = out + b_out[None, None, :]
    return out.astype(np.float32), res


def kernel(x, W_qkv, W_out, b_out):
    out, _ = run(x, W_qkv, W_out, b_out, trace=False)
    return out
